# revision 1
# baseline (speedup 1.0000x reference)
"""GCN2Conv (variant=False) Trainium2 kernel.

out = beta * (support @ theta) + (1-beta) * support
support = (1-alpha) * (D^-1/2 (A+I) D^-1/2 @ x) + alpha * h0
beta = log(lamda/l + 1)

Sharding: B=4 graphs over 8 cores -> 2 cores per graph, each owning
m_rows = N/2 = 1500 adjacency rows. x is replicated within a pair, theta
replicated everywhere. The bmm is a local row-block matmul (no cross-device
reduce); only the degree vector (row sums of A+I, needed for the D^-1/2
column scaling of x) is exchanged with a tiny per-pair AllGather.

Device pipeline per core:
  Phase 1 (DMA bound): stream adj row-tiles [128, N]; ACT computes row sums
    via activation(accum_out=...); PE transposes each [128,128] block into
    PSUM; DVE/ACT copy them (cast to bf16) into an SBUF-resident A^T.
  Degree exchange: AllGather [1500] -> [3000] within each pair.
  Phase 2: hi^T = (D x)^T A^T with xs stationary per k-block (N=512 fp32
    PSUM accumulation), fused epilogue in transposed layout, theta matmul,
    transpose back, store.
"""

import math
import sys

import numpy as np

sys.path.insert(0, "/opt/trn_rl_repo")

import concourse.bacc as bacc
import concourse.mybir as mybir
import concourse.tile as tile
from concourse import bass_utils, masks
from concourse.mybir import dt

AF = mybir.ActivationFunctionType

F = 128          # feature dim (= theta size), fixed
P = 128          # SBUF partitions
CHUNK = 512      # phase-2 m-chunk width (one fp32 PSUM bank)

B_FULL, N_FULL = 4, 3000
N_CORES_FULL = 8
M_FULL = N_FULL // 2


def _tile_sizes(total, step):
    return [min(step, total - s) for s in range(0, total, step)]


def build_program(n_nodes, m_rows, n_cores, alpha, beta, at_dtype=dt.bfloat16,
                  debug_dump=False):
    """Build the SPMD Bass program (identical on every core).

    Per-core external inputs (host pre-slices):
      adj_rows [m_rows, n_nodes], x_full [n_nodes, F], x_loc [m_rows, F],
      h0_loc [m_rows, F], theta [F, F].
    Output: out [m_rows, F].
    Cores 2g, 2g+1 own rows [0:m_rows], [m_rows:2*m_rows] of graph g.
    """
    assert n_nodes == 2 * m_rows
    c1 = 1.0 - alpha

    KT = math.ceil(n_nodes / P)        # k blocks (adj cols / nodes)
    kw = _tile_sizes(n_nodes, P)
    MT = math.ceil(m_rows / P)         # local m tiles
    mh = _tile_sizes(m_rows, P)
    mfull, mtail = m_rows // P, m_rows % P
    kfull, ktail = n_nodes // P, n_nodes % P
    # phase-2 chunks: groups of up to 4 full m-tiles (512 cols) or the tail
    # tile alone -- each chunk is one contiguous piece of A^T and gets its
    # own PSUM accumulation bank
    mchunks = []
    ti = 0
    while ti < MT:
        if mh[ti] == P:
            tj = ti
            while tj < MT and mh[tj] == P and tj - ti < 4:
                tj += 1
            mchunks.append((ti * P, (tj - ti) * P, ti, tj, P))
            ti = tj
        else:
            mchunks.append((ti * P, mh[ti], ti, ti + 1, mh[ti]))
            ti += 1

    nc = bacc.Bacc(
        "TRN2", target_bir_lowering=False, debug=False, num_devices=n_cores
    )
    adj = nc.dram_tensor("adj_rows", [m_rows, n_nodes], dt.float32, kind="ExternalInput")
    x_full = nc.dram_tensor("x_full", [n_nodes, F], dt.float32, kind="ExternalInput")
    x_loc = nc.dram_tensor("x_loc", [m_rows, F], dt.float32, kind="ExternalInput")
    h0_loc = nc.dram_tensor("h0_loc", [m_rows, F], dt.float32, kind="ExternalInput")
    theta = nc.dram_tensor("theta", [F, F], dt.float32, kind="ExternalInput")
    out_d = nc.dram_tensor("out", [m_rows, F], dt.float32, kind="ExternalOutput")

    groups = [[2 * g, 2 * g + 1] for g in range(n_cores // 2)]

    with tile.TileContext(nc) as tc:
        from contextlib import ExitStack

        with ExitStack() as ctx:
            ep = ctx.enter_context

            consts = ep(tc.tile_pool(name="consts", bufs=1))
            at_pool = ep(tc.tile_pool(name="at", bufs=1))
            nat_pool = ep(tc.tile_pool(name="nat", bufs=3))
            scr_pool = ep(tc.tile_pool(name="scr", bufs=4))
            deg_pool = ep(tc.tile_pool(name="deg", bufs=1))
            xs_pool = ep(tc.tile_pool(name="xs", bufs=1))
            tvec_pool = ep(tc.tile_pool(name="tvec", bufs=1))
            stream_pool = ep(tc.tile_pool(name="stream", bufs=2))
            sup_pool = ep(tc.tile_pool(name="sup", bufs=2))
            outc_pool = ep(tc.tile_pool(name="outc", bufs=2))
            outt_pool = ep(tc.tile_pool(name="outt", bufs=2))
            ptx_pool = ep(tc.tile_pool(name="ptx", bufs=2, space="PSUM"))
            dram = ep(tc.tile_pool(name="dram", bufs=1, space="DRAM"))

            ident = consts.tile([P, P], dt.float32)
            masks.make_identity(nc, ident[:])

            theta_sb = consts.tile([F, F], dt.float32)
            nc.sync.dma_start(theta_sb[:], theta[:])
            thetaB = consts.tile([F, F], dt.float32)
            nc.vector.tensor_scalar_mul(thetaB[:], theta_sb[:], beta)

            # A^T resident in SBUF: [k_local, (m_tile, kb, m_local)] -- one
            # contiguous [KT, 128] region per m-tile so the blocked xbar
            # transpose writes it in a single instruction
            AT = at_pool.tile([P, MT * KT * P], at_dtype)
            AT4 = AT[:].rearrange("p (i kb m) -> p i kb m", i=MT, kb=KT)

            # local degree accumulator: col i = row sums of local m-tile i
            deg_sb = deg_pool.tile([P, MT], dt.float32)
            nc.gpsimd.memset(deg_sb[:], 1.0)  # garbage lanes stay rsqrt-safe

            # ---------------- Phase 1: stream adj, rowsum + transpose ----------
            # SWDGE plain fp32 half-tile loads; ACT casts to bf16 + row-sums
            # (two halves, accumulators summed later); one blocked xbar
            # transpose per m-tile from the bf16 tile into A^T.
            from concourse.tile import add_dep_helper as _adh

            deg_sbB = deg_pool.tile([P, MT], dt.float32, tag="degB")
            nc.gpsimd.memset(deg_sbB[:], 1.0)
            HALF = KT * P // 2  # columns per load half (KT is even or padded)
            half_w = [min(HALF, n_nodes), max(0, n_nodes - HALF)]

            # The xbar-transpose's data accesses are invisible to Tile's dep
            # tracker, so fence manually:
            #  - RAW: transpose waits the two ACT cast+rowsum ops that write
            #    its bf16 source tile
            #  - WAR: the ACTs reusing a bf16 slot depend on the transpose
            #    that last read it (HWDGE producer -> waits its DMA lane)
            t_insts = []
            NAT16_BUFS = 3
            for i in range(MT):
                h = mh[i]
                nat16 = nat_pool.tile([P, KT * P], at_dtype, tag="nat16")
                acts = []
                for hf in range(2):
                    wcol = half_w[hf]
                    natf = scr_pool.tile([P, HALF], dt.float32, tag="natf")
                    nc.sync.dma_start(
                        natf[:h, 0:wcol],
                        adj[P * i : P * i + h, hf * HALF : hf * HALF + wcol],
                    )
                    dst = deg_sb if hf == 0 else deg_sbB
                    act = nc.scalar.activation(
                        nat16[:h, hf * HALF : hf * HALF + wcol],
                        natf[:h, 0:wcol],
                        AF.Copy,
                        accum_out=dst[:h, i : i + 1],
                    )
                    if i >= NAT16_BUFS:
                        _adh(act.ins, t_insts[i - NAT16_BUFS].ins, sync=True,
                             reason="nat16 slot WAR vs xbar transpose")
                    acts.append(act)

                # one blocked transpose for the whole row-tile:
                # in [128, KT*128] -> out [128, KT, 128] (3D out folds kb into
                # the logical partition dim; out region contiguous). Tail
                # tiles read/write garbage rows beyond h -- never consumed.
                t_inst = nc.sync.dma_start_transpose(
                    AT4[:, i, :, :],
                    nat16[:P, 0 : KT * P],
                )
                for act in acts:
                    _adh(t_inst.ins, act.ins, sync=True,
                         reason="xbar transpose RAW fence via ACT cast")
                t_insts.append(t_inst)

            # ---------------- degree: +1 self loop, pair exchange --------------
            degp = deg_pool.tile([P, MT], dt.float32)
            nc.vector.tensor_add(degp[:], deg_sb[:], deg_sbB[:])
            nc.vector.tensor_scalar_add(degp[:], degp[:], 1.0)

            degT_ps = ptx_pool.tile([P, P], dt.float32, tag="sm")
            nc.tensor.transpose(degT_ps[:MT, :P], degp[:P, :MT], ident[:P, :P])
            degT = deg_pool.tile([MT, P], dt.float32)
            nc.vector.tensor_copy(degT[:], degT_ps[:MT, :P])

            deg_loc_d = dram.tile([m_rows], dt.float32)
            deg_full_d = dram.tile([n_nodes], dt.float32)
            if mfull:
                nc.gpsimd.dma_start(
                    deg_loc_d[0 : mfull * P].rearrange("(a b) -> a b", b=P),
                    degT[0:mfull, :],
                )
            if mtail:
                nc.gpsimd.dma_start(
                    deg_loc_d[mfull * P : m_rows].rearrange("(a b) -> a b", a=1),
                    degT[mfull : mfull + 1, 0:mtail],
                )
            # xbar-mode transposes must not run concurrently with the
            # collective's DMAs (HW deadlock) and phase 2 must see completed
            # A^T -- gate on the transpose-completion semaphore.
            ag = nc.gpsimd.collective_compute(
                "AllGather",
                mybir.AluOpType.bypass,
                replica_groups=groups,
                ins=[deg_loc_d[:]],
                outs=[deg_full_d[:]],
            )
            _adh(ag.ins, t_insts[-1].ins, sync=True,
                 reason="xbar-vs-collective serialization")

            # PE HAM warm-up: dummy matmuls right after the AllGather so the
            # phase-2 matmuls start at the warm 2.4 GHz clock
            with tc.tile_pool(name="warm_ps", bufs=1, space="PSUM") as warm_pool:
                wp = warm_pool.tile([P, CHUNK], dt.float32)
                n_warm = 18
                for j in range(n_warm):
                    wmm = nc.tensor.matmul(
                        wp[:P, 0:CHUNK],
                        AT[:P, 0:P],
                        AT[:P, 0:CHUNK],
                        start=(j == 0),
                        stop=(j == n_warm - 1),
                    )
                    if j == 0:
                        warm0 = wmm

            # local row-scale vector in free-aligned layout; read back from the
            # DRAM copy (avoids SBUF->SBUF DMA, which deadlocks vs xbar mode)
            vecs = tvec_pool.tile([P, m_rows], dt.float32)
            deg_row = vecs[0:1, :]
            nc.gpsimd.dma_start(
                deg_row[0:1, 0:m_rows],
                deg_loc_d[:].rearrange("(a b) -> a b", a=1),
            )
            dis_row = vecs[0:1, :]
            nc.vector.reciprocal(dis_row, deg_row)
            nc.scalar.sqrt(dis_row, dis_row)

            # broadcast dis across partitions, then rs = c1*dis, s1 = c1*dis^2
            s1_b = tvec_pool.tile([P, m_rows], dt.float32, tag="s1_b")
            nc.gpsimd.partition_broadcast(s1_b[:], dis_row)
            rs_b = tvec_pool.tile([P, m_rows], dt.float32, tag="rs_b")
            nc.vector.tensor_scalar_mul(rs_b[:], s1_b[:], c1)
            nc.vector.tensor_mul(s1_b[:], s1_b[:], rs_b[:])

            # global degrees -> dis per k-block [P, KT]
            dgT = deg_pool.tile([P, P], dt.float32, tag="dgT")
            nc.gpsimd.memset(dgT[:KT, :], 1.0)
            dg_lds = []
            if kfull:
                dg_lds.append(nc.gpsimd.dma_start(
                    dgT[0:kfull, 0:P],
                    deg_full_d[0 : kfull * P].rearrange("(a b) -> a b", b=P),
                ))
            if ktail:
                dg_lds.append(nc.gpsimd.dma_start(
                    dgT[kfull : kfull + 1, 0:ktail],
                    deg_full_d[kfull * P : n_nodes],
                ))
            # anchor the PE warm-up on the first post-AG data load so the
            # warm clock carries into the phase-2 matmuls
            for dl in dg_lds[:1]:
                _adh(warm0.ins, dl.ins, sync=True, reason="warmup after AG data")
            dg_ps = ptx_pool.tile([P, P], dt.float32, tag="sm")
            nc.tensor.transpose(dg_ps[:P, :KT], dgT[:KT, :P], ident[:KT, :KT])
            disg = deg_pool.tile([P, KT], dt.float32)
            nc.vector.tensor_copy(disg[:], dg_ps[:P, :KT])
            nc.vector.reciprocal(disg[:], disg[:])
            nc.scalar.sqrt(disg[:], disg[:])

            # xs = D^-1/2 x in [k_local, (kb, f)] layout, cast to at_dtype
            xg = xs_pool.tile([P, KT * F], at_dtype)
            if kfull:
                nc.gpsimd.dma_start(
                    xg[:].rearrange("p (kb f) -> p kb f", kb=KT)[:, 0:kfull, :],
                    x_full[0 : kfull * P, :].rearrange("(kb p) f -> p kb f", p=P),
                )
            if ktail:
                nc.gpsimd.dma_start(
                    xg[0:ktail, kfull * F : (kfull + 1) * F],
                    x_full[kfull * P : n_nodes, :],
                )
            xs = xs_pool.tile([P, KT * F], at_dtype)
            for kb in range(KT):
                w = kw[kb]
                nc.vector.tensor_scalar_mul(
                    xs[:w, kb * F : kb * F + F],
                    xg[:w, kb * F : kb * F + F],
                    disg[:w, kb : kb + 1],
                )

            # x_loc / h0_loc transposed: xT [f, m], h0aT = alpha * h0^T
            # (bulk SWDGE loads in the per-m-tile [p, (i f)] layout)
            xT = xs_pool.tile([P, m_rows], dt.float32, tag="xT")
            h0aT = xs_pool.tile([P, m_rows], dt.float32, tag="h0aT")
            xn_all = xs_pool.tile([P, MT * F], dt.float32, tag="xn_all")
            hn_all = xs_pool.tile([P, MT * F], dt.float32, tag="hn_all")
            for src, dst in ((x_loc, xn_all), (h0_loc, hn_all)):
                if mfull:
                    nc.gpsimd.dma_start(
                        dst[:].rearrange("p (i f) -> p i f", i=MT)[:, 0:mfull, :],
                        src[0 : mfull * P, :].rearrange("(i p) f -> p i f", p=P),
                    )
                if mtail:
                    nc.gpsimd.dma_start(
                        dst[0:mtail, mfull * F : (mfull + 1) * F],
                        src[mfull * P : m_rows, :],
                    )
            for i in range(MT):
                h = mh[i]
                xt_ps = ptx_pool.tile([P, P], dt.float32, tag="sm")
                nc.tensor.transpose(
                    xt_ps[:F, :h], xn_all[:h, i * F : i * F + F], ident[:h, :h]
                )
                nc.vector.tensor_copy(xT[:, P * i : P * i + h], xt_ps[:F, :h])

                ht_ps = ptx_pool.tile([P, P], dt.float32, tag="sm")
                nc.tensor.transpose(
                    ht_ps[:F, :h], hn_all[:h, i * F : i * F + F], ident[:h, :h]
                )
                nc.scalar.activation(
                    h0aT[:, P * i : P * i + h], ht_ps[:F, :h], AF.Copy, scale=alpha
                )

            # qT = s1 * x^T + alpha * h0^T  (everything but the hi term)
            qT = xs_pool.tile([P, m_rows], dt.float32, tag="qT")
            nc.vector.tensor_mul(qT[:], xT[:], s1_b[:])
            nc.vector.tensor_add(qT[:], qT[:], h0aT[:])

            if debug_dump:
                dbg_at = nc.dram_tensor(
                    "dbg_at", [P, MT * KT * P], at_dtype, kind="ExternalOutput"
                )
                dbg_dma = nc.sync.dma_start(dbg_at[:], AT[:])
                _adh(dbg_dma.ins, t_insts[-1].ins, sync=True, reason="dbg")
                dbg_xs = nc.dram_tensor(
                    "dbg_xs", [P, KT * F], at_dtype, kind="ExternalOutput"
                )
                nc.sync.dma_start(dbg_xs[:], xs[:])

            # ---------------- Phase 2: matmuls + epilogue ----------------------
            with tc.tile_pool(name="hi_ps", bufs=2, space="PSUM") as hi_pool, \
                 tc.tile_pool(name="o2_ps", bufs=2, space="PSUM") as o2_pool:
                for s, wc, ia, ib, tw in mchunks:
                    hiT = hi_pool.tile([P, CHUNK], dt.float32)
                    for kb in range(KT):
                        w = kw[kb]
                        nc.tensor.matmul(
                            hiT[:F, 0:wc],
                            xs[:w, kb * F : kb * F + F],
                            AT4[:w, ia:ib, kb, 0:tw],
                            start=(kb == 0),
                            stop=(kb == KT - 1),
                        )
                    supT = sup_pool.tile([P, CHUNK], dt.float32)
                    nc.vector.tensor_mul(supT[:, 0:wc], hiT[:F, 0:wc], rs_b[:, s : s + wc])
                    nc.vector.tensor_add(supT[:, 0:wc], supT[:, 0:wc], qT[:, s : s + wc])

                    o2T = o2_pool.tile([P, CHUNK], dt.float32)
                    nc.tensor.matmul(
                        o2T[:F, 0:wc], thetaB[:F, :F], supT[:F, 0:wc],
                        start=True, stop=True,
                    )
                    outT = outc_pool.tile([P, CHUNK], dt.float32)
                    nc.vector.scalar_tensor_tensor(
                        outT[:, 0:wc], supT[:, 0:wc], 1.0 - beta, o2T[:F, 0:wc],
                        mybir.AluOpType.mult, mybir.AluOpType.add,
                    )

                    # back to natural [m, f] and store
                    for off in range(0, wc, P):
                        hh = min(P, wc - off)
                        ot_ps = ptx_pool.tile([P, P], dt.float32, tag="sm")
                        nc.tensor.transpose(
                            ot_ps[:hh, :F], outT[:F, off : off + hh], ident[:F, :F]
                        )
                        ot = outt_pool.tile([P, F], dt.float32)
                        nc.vector.tensor_copy(ot[:hh, :], ot_ps[:hh, :F])
                        nc.sync.dma_start(
                            out_d[s + off : s + off + hh, :], ot[:hh, :]
                        )

    nc.compile()
    return nc


def _copy(eng, out_ap, in_ap):
    if hasattr(eng, "tensor_copy"):
        eng.tensor_copy(out_ap, in_ap)
    else:
        eng.copy(out_ap, in_ap)


def make_in_maps(x, adj, h0, theta, n_cores):
    m = x.shape[1] // 2
    in_maps = []
    for c in range(n_cores):
        b, half = c // 2, c % 2
        r0 = half * m
        in_maps.append(
            {
                "adj_rows": adj[b, r0 : r0 + m, :],
                "x_full": x[b],
                "x_loc": x[b, r0 : r0 + m, :],
                "h0_loc": h0[b, r0 : r0 + m, :],
                "theta": theta,
            }
        )
    return in_maps


_CACHE = {}


def _get_program(key, *args, **kwargs):
    if key not in _CACHE:
        _CACHE[key] = build_program(*args, **kwargs)
    return _CACHE[key]


def kernel(x, adj, h0, theta, lamda, alpha, l):
    x = np.asarray(x, dtype=np.float32)
    adj = np.asarray(adj, dtype=np.float32)
    h0 = np.asarray(h0, dtype=np.float32)
    theta = np.asarray(theta, dtype=np.float32)
    lamda_f = float(np.asarray(lamda))
    alpha_f = float(np.asarray(alpha))
    l_f = float(np.asarray(l))
    beta_f = float(math.log(lamda_f / l_f + 1.0))

    B, N, Fdim = x.shape
    assert (B, N, Fdim) == (B_FULL, N_FULL, F)
    M = N // 2

    nc = _get_program(
        ("full", alpha_f, beta_f), N, M, N_CORES_FULL, alpha_f, beta_f
    )

    in_maps = make_in_maps(x, adj, h0, theta, N_CORES_FULL)
    res = bass_utils.run_bass_kernel_spmd(
        nc, in_maps, list(range(N_CORES_FULL))
    ).results

    out = np.empty((B, N, Fdim), dtype=np.float32)
    for c in range(N_CORES_FULL):
        b, half = c // 2, c % 2
        out[b, half * M : (half + 1) * M, :] = res[c]["out"]
    return out



# revision 8
# speedup vs baseline: 2.4537x; 2.4537x over previous
"""GCN2Conv (variant=False) Trainium2 kernel, v2.

Math (all linear, so theta folds out of the critical path):
  out = support @ T',              T' = beta*theta + (1-beta)*I
  support = c1*hi + alpha*h0,      c1 = 1-alpha
  hi = dis_r . ((A+I) @ (dis . x)),  dis = (rowsum(A)+1)^-1/2
=>
  out[R] = dis[R] . (A_R @ Gd) + dis[R]^2 . G[R] + H[R]
  G = c1 * (x @ T'),  Gd = dis . G,  H = alpha * (h0[R] @ T')

Sharding: B=4 graphs x 2 cores/graph. Core pair (2g, 2g+1) owns rows
[0:1536) / [1536:3000) of graph g (128-aligned halves; everything zero
padded to N_PAD=3072, M_PAD=1536 so the SPMD program is identical on
both halves).

Host passes pure layout transforms only (slice / transpose / pad / fp16
cast): AT = A[R,:].T as [n_chunks, KT, 128, CHUNK] fp16 so stream DMAs
are contiguous; xT_loc = x[R].T, h0T = h0[R].T fp16; x_full fp16;
theta fp32. Output is produced transposed [F, M_PAD] fp32 and the host
transposes it back while unsharding.

Device pipeline per core:
  - 3 sub-streams (one per 512-wide m-chunk of A^T), 4 DMAs each.
  - PE ones-vector matmuls reduce A^T over partitions per chunk =
    row degrees of own rows; +1 self loop on copy-out.
  - 3 pipelined pair-AllGathers (2KB each) exchange degree chunks; each
    unlocks a "wave" of 8 k-blocks (4 even-side + 4 odd-side).
  - Gd = dis . G per k-block; main matmul rawT[f,m] += Gd_kb^T AT_kb
    accumulates 24 k-blocks into 3 PSUM banks, emitted interleaved with
    the stream-chasing rowsums (PE executes in order).
  - Epilogue per chunk: outT = rawT . dis_R + (G_R/deg_R + H)^T; one
    768KB store of outT [128, 1536] fp32.
"""

import math
import sys

import numpy as np

sys.path.insert(0, "/opt/trn_rl_repo")

import concourse.bacc as bacc
import concourse.mybir as mybir
import concourse.tile as tile
from concourse import bass_utils, masks
from concourse.mybir import dt

AF = mybir.ActivationFunctionType

F = 128            # feature dim
P = 128            # SBUF partitions

B_FULL, N_FULL = 4, 3000
N_CORES_FULL = 8
M_PAD_FULL = 1536          # even core rows [0:1536), odd [1536:3000)
N_PAD_FULL = 3072
CHUNK_FULL = 512
NCH = 3                    # m-chunks / AG waves (schedule hardcoded for 3)


def build_program(n_pad, m_pad, chunk, n_cores, alpha, beta, n_quarters=4):
    """Build the SPMD Bass program (identical on every core).

    Per-core inputs:
      adjT  [nch*KT*128, chunk] f16 : A[R,:].T padded, chunk-major
      x_full [n_pad, F] f16, xT_loc [F, m_pad] f16, h0T [F, m_pad] f16,
      theta [F, F] f32.
    Output: outT [F, m_pad] f32 (transposed).
    """
    assert n_pad == 2 * m_pad and m_pad % chunk == 0 and chunk % P == 0
    KT = n_pad // P                 # k tiles (contraction blocks)
    nch = m_pad // chunk            # m-chunks == AG waves
    assert nch == NCH
    K = chunk // P                  # k-blocks unlocked per wave per side
    kb_odd = m_pad // P             # first odd-side k-block
    assert KT % n_quarters == 0
    kb_per_q = KT // n_quarters
    c1 = 1.0 - alpha

    # wave_c k-blocks: even rows [c*K, (c+1)*K) + odd rows shifted
    waves = [list(range(c * K, (c + 1) * K))
             + list(range(kb_odd + c * K, kb_odd + (c + 1) * K))
             for c in range(nch)]

    nc = bacc.Bacc(
        "TRN2", target_bir_lowering=False, debug=False, num_devices=n_cores
    )
    adjT = nc.dram_tensor(
        "adjT", [nch * KT * P, chunk], dt.float16, kind="ExternalInput"
    )
    x_full = nc.dram_tensor("x_full", [n_pad, F], dt.float16, kind="ExternalInput")
    xT_loc = nc.dram_tensor("xT_loc", [F, m_pad], dt.float16, kind="ExternalInput")
    h0T = nc.dram_tensor("h0T", [F, m_pad], dt.float16, kind="ExternalInput")
    theta = nc.dram_tensor("theta", [F, F], dt.float32, kind="ExternalInput")
    outT_d = nc.dram_tensor("outT", [F, m_pad], dt.float32, kind="ExternalOutput")

    groups = [[2 * g, 2 * g + 1] for g in range(max(1, n_cores // 2))]

    with tile.TileContext(nc) as tc:
        from contextlib import ExitStack

        with ExitStack() as ctx:
            ep = ctx.enter_context

            consts = ep(tc.tile_pool(name="consts", bufs=1))
            at_pool = ep(tc.tile_pool(name="at", bufs=1))
            xs_pool = ep(tc.tile_pool(name="xs", bufs=1))
            deg_pool = ep(tc.tile_pool(name="deg", bufs=1))
            out_pool = ep(tc.tile_pool(name="out", bufs=1))
            ps_raw = ep(tc.tile_pool(name="ps_raw", bufs=3, space="PSUM"))
            ps_deg = ep(tc.tile_pool(name="ps_deg", bufs=2, space="PSUM"))
            ps_sm = ep(tc.tile_pool(name="ps_sm", bufs=2, space="PSUM"))
            dram = ep(tc.tile_pool(name="dram", bufs=1, space="DRAM"))

            # ---------------- constants -----------------------------------
            ident = consts.tile([P, P], dt.float32)
            masks.make_identity(nc, ident[:])
            ident16 = consts.tile([P, P], dt.float16)
            nc.vector.tensor_copy(ident16[:], ident[:])
            ones = consts.tile([P, 1], dt.float16)
            nc.gpsimd.memset(ones[:], 1.0)

            theta_sb = consts.tile([F, F], dt.float32)
            nc.scalar.dma_start(theta_sb[:], theta[:])
            # T' = beta*theta + (1-beta)*I ; thG = c1*T' ; thH = alpha*T'
            thetaP = consts.tile([F, F], dt.float32)
            nc.vector.tensor_scalar_mul(thetaP[:], theta_sb[:], beta)
            nc.vector.scalar_tensor_tensor(
                thetaP[:], ident[:], 1.0 - beta, thetaP[:],
                mybir.AluOpType.mult, mybir.AluOpType.add,
            )
            thG = consts.tile([F, F], dt.float16)
            nc.vector.tensor_scalar_mul(thG[:], thetaP[:], c1)
            thH = consts.tile([F, F], dt.float16)
            nc.vector.tensor_scalar_mul(thH[:], thetaP[:], alpha)

            # ---------------- bulk loads -----------------------------------
            # A^T resident: free layout (chunk, kb, m) fp16
            AT = at_pool.tile([P, nch * KT * chunk], dt.float16)
            AT4 = AT[:].rearrange("p (c kb m) -> p c kb m", c=nch, kb=KT)
            adjT_ap = adjT[:].rearrange(
                "(c kb p) m -> p c kb m", c=nch, kb=KT, p=P
            )

            def emit_stream(c, q):
                k0 = q * kb_per_q
                nc.sync.dma_start(
                    AT4[:, c, k0 : k0 + kb_per_q, :],
                    adjT_ap[:, c, k0 : k0 + kb_per_q, :],
                )

            for c in range(nch):
                for q in range(n_quarters):
                    emit_stream(c, q)

            # x in per-k-tile layout [p, kb, f]
            xg = xs_pool.tile([P, KT * F], dt.float16)
            nc.scalar.dma_start(
                xg[:].rearrange("p (kb f) -> p kb f", kb=KT),
                x_full[:].rearrange("(kb p) f -> p kb f", p=P),
            )
            xTl = xs_pool.tile([P, m_pad], dt.float16, tag="xTl")
            nc.scalar.dma_start(xTl[:], xT_loc[:])
            h0T_sb = xs_pool.tile([P, m_pad], dt.float16, tag="h0T")
            nc.scalar.dma_start(h0T_sb[:], h0T[:])

            # ---------------- degree rowsums (PE partition-reduce) ---------
            degrow = deg_pool.tile([1, m_pad], dt.float32)
            rcp = deg_pool.tile([1, m_pad], dt.float32, tag="rcp")
            rs_row = deg_pool.tile([1, m_pad], dt.float32, tag="rs_row")
            rs_b = deg_pool.tile([P, m_pad], dt.float32, tag="rs_b")
            deg_ps_tiles = {}

            def emit_rowsums(c, q):
                if q == 0:
                    deg_ps_tiles[c] = ps_deg.tile(
                        [1, chunk], dt.float32, name=f"deg_ps_{c}",
                        tag="degps", bufs=2,
                    )
                dps = deg_ps_tiles[c]
                for kb in range(q * kb_per_q, (q + 1) * kb_per_q):
                    nc.tensor.matmul(
                        dps[0:1, :], ones[:, 0:1], AT4[:, c, kb, :],
                        start=(kb == 0), stop=(kb == KT - 1),
                    )

            def emit_deg_chunk_post(c):
                # +1 self loop on psum -> sbuf copy; local dis pieces
                s = c * chunk
                dps = deg_ps_tiles[c]
                nc.vector.tensor_scalar_add(
                    degrow[0:1, s : s + chunk], dps[0:1, :], 1.0
                )
                nc.vector.reciprocal(
                    rcp[0:1, s : s + chunk], degrow[0:1, s : s + chunk]
                )
                nc.scalar.sqrt(
                    rs_row[0:1, s : s + chunk], rcp[0:1, s : s + chunk]
                )
                nc.gpsimd.partition_broadcast(
                    rs_b[:, s : s + chunk], rs_row[0:1, s : s + chunk]
                )

            # ---------------- x^T transposes + G ---------------------------
            xT = xs_pool.tile([P, KT * F], dt.float16, tag="xT")
            G = xs_pool.tile([P, KT * F], dt.float16, tag="G")
            Gd = xs_pool.tile([P, KT * F], dt.float16, tag="Gd")

            def emit_xt_g(kb):
                tp = ps_sm.tile([P, P], dt.float16, tag="sm")
                nc.tensor.transpose(
                    tp[:P, :P], xg[:, kb * F : (kb + 1) * F], ident16[:P, :P]
                )
                nc.scalar.activation(
                    xT[:, kb * F : (kb + 1) * F], tp[:P, :P], AF.Copy
                )
                gp = ps_sm.tile([P, F], dt.float32, tag="sm")
                nc.tensor.matmul(
                    gp[:P, :F], xT[:, kb * F : (kb + 1) * F], thG[:, :],
                    start=True, stop=True,
                )
                nc.scalar.activation(
                    G[:, kb * F : (kb + 1) * F], gp[:P, :F], AF.Copy
                )

            # ---------------- QT = (G_R / deg_R + H)^T ---------------------
            QT = out_pool.tile([P, m_pad], dt.float32, tag="QT")
            GoT = out_pool.tile([P, m_pad], dt.float16, tag="GoT")
            rs2_b = deg_pool.tile([P, chunk], dt.float32, tag="rs2_b")

            def emit_goh(c):
                s = c * chunk
                hp = ps_sm.tile([P, chunk], dt.float32, tag="sm")
                nc.tensor.matmul(
                    hp[:F, :chunk], thH[:, :], h0T_sb[:, s : s + chunk],
                    start=True, stop=True,
                )
                nc.scalar.activation(QT[:, s : s + chunk], hp[:F, :chunk], AF.Copy)
                gp2 = ps_sm.tile([P, chunk], dt.float32, tag="sm")
                nc.tensor.matmul(
                    gp2[:F, :chunk], thG[:, :], xTl[:, s : s + chunk],
                    start=True, stop=True,
                )
                nc.scalar.activation(GoT[:, s : s + chunk], gp2[:F, :chunk], AF.Copy)

            def emit_qt(c):
                # QT += GoT * (1/deg) broadcast
                s = c * chunk
                nc.gpsimd.partition_broadcast(rs2_b[:, :], rcp[0:1, s : s + chunk])
                tmp = deg_pool.tile([P, chunk], dt.float32, tag="qtmp")
                nc.vector.tensor_mul(tmp[:, :], GoT[:, s : s + chunk], rs2_b[:, :])
                nc.vector.tensor_add(
                    QT[:, s : s + chunk], QT[:, s : s + chunk], tmp[:, :]
                )

            # ---------------- degree exchange (pipelined AGs) --------------
            deg_loc_d = dram.tile([m_pad], dt.float32)
            deg_pair_d = [dram.tile([2 * chunk], dt.float32, tag=f"dp{c}",
                                    name=f"deg_pair_{c}")
                          for c in range(nch)]
            disg = deg_pool.tile([P, KT], dt.float32, tag="disg")

            def emit_ag(c):
                s = c * chunk
                nc.scalar.dma_start(
                    deg_loc_d[s : s + chunk].rearrange("(a m) -> a m", a=1),
                    degrow[0:1, s : s + chunk],
                )
                nc.gpsimd.collective_compute(
                    "AllGather",
                    mybir.AluOpType.bypass,
                    replica_groups=groups,
                    ins=[deg_loc_d[s : s + chunk]],
                    outs=[deg_pair_d[c][:]],
                )

            def emit_wave_dis(c):
                # AG output: [even chunk degs | odd chunk degs] -> disg cols
                for side in range(2):
                    dg = deg_pool.tile([K, P], dt.float32, tag="dgT", bufs=2)
                    nc.scalar.dma_start(
                        dg[:, :],
                        deg_pair_d[c][side * chunk : (side + 1) * chunk]
                        .rearrange("(a b) -> a b", b=P),
                    )
                    tp = ps_sm.tile([P, K], dt.float32, tag="sm")
                    nc.tensor.transpose(tp[:P, :K], dg[:K, :P], ident[:K, :K])
                    kb0 = side * kb_odd + c * K
                    nc.vector.reciprocal(disg[:, kb0 : kb0 + K], tp[:P, :K])
                    nc.scalar.sqrt(disg[:, kb0 : kb0 + K], disg[:, kb0 : kb0 + K])

            def emit_gd(kbs):
                for kb in kbs:
                    nc.vector.tensor_scalar_mul(
                        Gd[:, kb * F : (kb + 1) * F],
                        G[:, kb * F : (kb + 1) * F],
                        disg[:, kb : kb + 1],
                    )

            # ---------------- main matmul + epilogue ------------------------
            raw_ps = [ps_raw.tile([P, chunk], dt.float32, name=f"raw_{c}",
                                  tag=f"raw{c}", bufs=1)
                      for c in range(nch)]
            n_mm_done = [0] * nch
            outT_sb = out_pool.tile([P, m_pad], dt.float32, tag="outT")

            def emit_mm(kbs, c):
                for kb in kbs:
                    nc.tensor.matmul(
                        raw_ps[c][:F, :chunk],
                        Gd[:, kb * F : (kb + 1) * F],
                        AT4[:, c, kb, :],
                        start=(n_mm_done[c] == 0),
                        stop=(n_mm_done[c] == KT - 1),
                    )
                    n_mm_done[c] += 1

            def emit_epilogue(c):
                s = c * chunk
                nc.vector.tensor_mul(
                    outT_sb[:, s : s + chunk], raw_ps[c][:F, :chunk],
                    rs_b[:, s : s + chunk],
                )
                nc.vector.tensor_add(
                    outT_sb[:, s : s + chunk], outT_sb[:, s : s + chunk],
                    QT[:, s : s + chunk],
                )

            # ---------------- emission schedule (PE is in-order) ------------
            # quarter q of chunk c lands ~ (4c+q+1)*2.2us; AG_c ~ stream of
            # chunk c + rowsums + AG latency. Interleave so PE never blocks
            # on a not-yet-landed quarter while ready work exists.
            for kb in range(KT):
                emit_xt_g(kb)
            for c in range(nch):
                emit_goh(c)
            for q in range(n_quarters):
                emit_rowsums(0, q)
            emit_deg_chunk_post(0)
            emit_qt(0)
            emit_ag(0)
            emit_rowsums(1, 0)
            emit_rowsums(1, 1)
            emit_wave_dis(0)
            emit_gd(waves[0])
            emit_mm(waves[0], 0)
            emit_rowsums(1, 2)
            emit_rowsums(1, 3)
            emit_deg_chunk_post(1)
            emit_qt(1)
            emit_ag(1)
            emit_mm(waves[0], 1)
            emit_rowsums(2, 0)
            emit_wave_dis(1)
            emit_gd(waves[1])
            emit_mm(waves[1], 0)
            emit_rowsums(2, 1)
            emit_mm(waves[1], 1)
            emit_rowsums(2, 2)
            # chunk-2 MMs for already-unlocked kbs, grouped by landed quarter
            ready01 = sorted(waves[0] + waves[1])
            emit_mm([kb for kb in ready01 if kb < 3 * kb_per_q], 2)
            emit_rowsums(2, 3)
            emit_deg_chunk_post(2)
            emit_qt(2)
            emit_ag(2)
            emit_mm([kb for kb in ready01 if kb >= 3 * kb_per_q], 2)
            emit_wave_dis(2)
            emit_gd(waves[2])
            emit_mm(waves[2], 0)
            emit_epilogue(0)
            emit_mm(waves[2], 1)
            emit_epilogue(1)
            emit_mm(waves[2], 2)
            emit_epilogue(2)
            nc.sync.dma_start(outT_d[:], outT_sb[:])

    nc.compile()
    return nc


def make_in_maps(x, adj, h0, theta, n_cores, n_pad, m_pad, chunk, n_real):
    KT = n_pad // P
    nch = m_pad // chunk
    f2 = np.float16
    in_maps = []
    x_pads = {}
    for c in range(n_cores):
        g, h = c // 2, c % 2
        r0 = 0 if h == 0 else m_pad
        m_real = m_pad if h == 0 else n_real - m_pad
        if g not in x_pads:
            xp = np.zeros((n_pad, F), f2)
            xp[:n_real] = x[g].astype(f2)
            x_pads[g] = xp
        at = np.zeros((n_pad, m_pad), f2)
        at[:n_real, :m_real] = adj[g, r0 : r0 + m_real, :].astype(f2).T
        at = np.ascontiguousarray(
            at.reshape(KT, P, nch, chunk).transpose(2, 0, 1, 3)
        ).reshape(nch * KT * P, chunk)
        xt = np.zeros((F, m_pad), f2)
        xt[:, :m_real] = x[g, r0 : r0 + m_real, :].astype(f2).T
        ht = np.zeros((F, m_pad), f2)
        ht[:, :m_real] = h0[g, r0 : r0 + m_real, :].astype(f2).T
        in_maps.append(
            {
                "adjT": at,
                "x_full": x_pads[g],
                "xT_loc": xt,
                "h0T": ht,
                "theta": theta.astype(np.float32),
            }
        )
    return in_maps


_CACHE = {}


def _get_program(key, *args, **kwargs):
    if key not in _CACHE:
        _CACHE[key] = build_program(*args, **kwargs)
    return _CACHE[key]


def kernel(x, adj, h0, theta, lamda, alpha, l):
    x = np.asarray(x, dtype=np.float32)
    adj = np.asarray(adj, dtype=np.float32)
    h0 = np.asarray(h0, dtype=np.float32)
    theta = np.asarray(theta, dtype=np.float32)
    lamda_f = float(np.asarray(lamda))
    alpha_f = float(np.asarray(alpha))
    l_f = float(np.asarray(l))
    beta_f = float(math.log(lamda_f / l_f + 1.0))

    B, N, Fdim = x.shape
    assert (B, N, Fdim) == (B_FULL, N_FULL, F)

    nc = _get_program(
        ("full", alpha_f, beta_f),
        N_PAD_FULL, M_PAD_FULL, CHUNK_FULL, N_CORES_FULL, alpha_f, beta_f,
    )

    in_maps = make_in_maps(
        x, adj, h0, theta, N_CORES_FULL,
        N_PAD_FULL, M_PAD_FULL, CHUNK_FULL, N_FULL,
    )
    res = bass_utils.run_bass_kernel_spmd(
        nc, in_maps, list(range(N_CORES_FULL))
    ).results

    out = np.empty((B, N, Fdim), dtype=np.float32)
    for c in range(N_CORES_FULL):
        g, h = c // 2, c % 2
        r0 = 0 if h == 0 else M_PAD_FULL
        m_real = M_PAD_FULL if h == 0 else N - M_PAD_FULL
        out[g, r0 : r0 + m_real, :] = res[c]["outT"][:, :m_real].T
    return out
